# revision 1
# baseline (speedup 1.0000x reference)
"""Trainium2 Bass kernel for nn_Loc2Cluster (GNN message passing, segment-max).

Computation: agg[c] = elementwise-max over locs with edge to cluster c of
x_locs[loc]; empty clusters -> 0; output = concat([x_clusters, agg], -1).

Strategy (cluster-sharded, zero collectives):
  - Core k owns clusters [4096k, 4096(k+1)).
  - Host routes each edge's loc row to the core owning its dst cluster.
  - Within a core, clusters are sorted by in-degree (desc). Rows are laid
    out in "rounds": round r holds the r-th edge row of every cluster with
    count > r, in sorted-cluster order. Sorted order makes each round a
    contiguous *prefix* of cluster slots, so the whole segment-max becomes
    ~max_degree elementwise tensor_max ops over shrinking prefixes -- no
    data-dependent addressing on device at all.
  - Round block layout is partition-major ([128, M_r/128, 256]) so every
    DMA is a plain contiguous copy and every cluster lives at a fixed
    (partition, chunk) slot of the SBUF accumulator.
  - Round 0 is DMA'd straight into the accumulator (tail slots for empty
    clusters are zero rows -> matches reference's 0-fill, no fixup pass).
  - Output [4096, 512] written per core: left half = x_clusters (sorted),
    right half = accumulator; host unsorts and stacks.
"""

import sys

import numpy as np

if "/opt/trn_rl_repo" not in sys.path:
    sys.path.insert(0, "/opt/trn_rl_repo")

N_LOCS = 262144
N_CLUSTERS = 32768
D = 256
N_CORES = 8
CPC = N_CLUSTERS // N_CORES  # 4096 clusters per core
P = 128
CHUNKS = CPC // P  # 32 chunks of 128 clusters
NEG = np.float32(-1e30)

LAST_RESULTS = None  # BassKernelResults of the most recent run (for profiling)
LAST_NC = None  # compiled Bass module of the most recent run (for TimelineSim)


def _host_prep(x_locs, x_clusters, edge_src, edge_dst):
    """Build per-core round-major row streams + sorted x_clusters shards."""
    x_locs = np.ascontiguousarray(np.asarray(x_locs, dtype=np.float32))
    x_clusters = np.ascontiguousarray(np.asarray(x_clusters, dtype=np.float32))
    src = np.asarray(edge_src).astype(np.int64)
    dst = np.asarray(edge_dst).astype(np.int64)
    n_edges = dst.shape[0]

    counts = np.bincount(dst, minlength=N_CLUSTERS)  # [32768]

    # Global order by count desc, dealt round-robin across cores: cluster
    # with global rank g goes to core g%8 at local rank g//8. This balances
    # the per-core round sizes to within 1 cluster, so the shared (SPMD)
    # round schedule has nearly zero cross-core padding, and each core's
    # local order is automatically count-sorted.
    gorder = np.argsort(-counts, kind="stable")  # [32768] cluster ids by rank
    grank = np.empty_like(gorder)
    grank[gorder] = np.arange(N_CLUSTERS)
    # order[k, s] = cluster id at core k local rank s
    order = np.ascontiguousarray(gorder.reshape(CPC, N_CORES).T)  # [8, CPC]

    # occurrence index of each edge within its dst cluster
    by_dst = np.argsort(dst, kind="stable")
    group_start = np.zeros(N_CLUSTERS, dtype=np.int64)
    np.cumsum(counts[:-1], out=group_start[1:])
    occ = np.empty(n_edges, dtype=np.int64)
    occ[by_dst] = np.arange(n_edges, dtype=np.int64) - group_start[dst[by_dst]]

    g_of = grank[dst]
    core_of = g_of % N_CORES
    rank_of = g_of // N_CORES

    # round schedule: m_r global = #clusters with count > r; per-core max
    # is ceil(m_r/8); round block padded to a multiple of 128 slots
    R = max(int(counts.max()), 1)
    counts_sorted = counts[gorder]
    m_r_g = (counts_sorted[None, :] > np.arange(R)[:, None]).sum(axis=1)
    m_r = (m_r_g + N_CORES - 1) // N_CORES  # per-core max
    M = ((m_r + P - 1) // P) * P
    M[0] = CPC  # round 0 covers every slot (zeros for empty clusters)
    offs = np.zeros(R + 1, dtype=np.int64)
    np.cumsum(M, out=offs[1:])
    TOT = int(offs[-1])

    # slot of each edge inside its core's stream (partition-major blocks)
    X = M // P  # chunks per round
    p_of = rank_of % P
    c_of = rank_of // P
    slot = offs[occ] + p_of * X[occ] + c_of

    slot_src = np.full((N_CORES, TOT), -1, dtype=np.int64)
    slot_src[core_of, slot] = src

    in_maps = []
    for k in range(N_CORES):
        ss = slot_src[k]
        stream = x_locs[np.maximum(ss, 0)]  # [TOT, 256]
        pad = ss < 0
        if pad[:CPC].any():
            stream[np.flatnonzero(pad[:CPC])] = 0.0  # empty clusters -> 0
        padr = np.flatnonzero(pad[CPC:]) + CPC
        if padr.size:
            stream[padr] = NEG  # later-round pads are max-neutral
        xc = x_clusters[order[k]]  # [CPC, D] by sorted rank
        xc = np.ascontiguousarray(
            xc.reshape(CHUNKS, P, D).transpose(1, 0, 2)
        )  # [P, CHUNKS, D]
        in_maps.append({"rows": np.ascontiguousarray(stream), "xc": xc})

    return in_maps, order, M, offs, TOT, x_clusters


def _build_program(R, M, offs, TOT, big_split=8, out_split=4, bufs=5):
    from concourse import bacc, mybir
    from concourse._compat import axon_active
    from concourse.tile import TileContext

    nc = bacc.Bacc(
        "TRN2",
        target_bir_lowering=False,
        debug=not axon_active(),
        num_devices=N_CORES,
    )
    rows_h = nc.dram_tensor("rows", [TOT, D], mybir.dt.float32, kind="ExternalInput")
    xc_h = nc.dram_tensor(
        "xc", [P, CHUNKS, D], mybir.dt.float32, kind="ExternalInput"
    )
    out_h = nc.dram_tensor(
        "out", [P, CHUNKS, 2 * D], mybir.dt.float32, kind="ExternalOutput"
    )

    with TileContext(nc) as tc:
        with (
            tc.tile_pool(name="accp", bufs=1) as accp,
            tc.tile_pool(name="stagep", bufs=bufs) as stagep,
        ):
            acc = accp.tile([P, CHUNKS * D], mybir.dt.float32)
            # round 0: DMA straight into the accumulator, split for
            # DMA-queue parallelism (each split is contiguous in HBM)
            r0 = rows_h.ap()[0:CPC].rearrange("(p x) f -> p (x f)", p=P)
            step = P // big_split
            for q in range(big_split):
                lo, hi = q * step, (q + 1) * step
                nc.sync.dma_start(out=acc[lo:hi, :], in_=r0[lo:hi, :])
            for r in range(1, R):
                Xr = int(M[r]) // P
                w = Xr * D
                blk = rows_h.ap()[int(offs[r]) : int(offs[r]) + int(M[r])].rearrange(
                    "(p x) f -> p (x f)", p=P
                )
                st = stagep.tile([P, CHUNKS * D], mybir.dt.float32, tag="stage")
                nsplit = big_split if Xr >= big_split else (4 if Xr >= 4 else 1)
                step = P // nsplit
                for q in range(nsplit):
                    lo, hi = q * step, (q + 1) * step
                    nc.sync.dma_start(out=st[lo:hi, :w], in_=blk[lo:hi, :])
                nc.vector.tensor_max(
                    out=acc[:, :w], in0=acc[:, :w], in1=st[:, :w]
                )
            # left half of output: x_clusters passthrough (DRAM->DRAM)
            step = P // out_split
            for q in range(out_split):
                lo, hi = q * step, (q + 1) * step
                nc.sync.dma_start(
                    out=out_h.ap()[lo:hi, :, 0:D], in_=xc_h.ap()[lo:hi]
                )
            # right half: the aggregated maxima
            acc3 = acc[:].rearrange("p (x f) -> p x f", f=D)
            for q in range(out_split):
                lo, hi = q * step, (q + 1) * step
                nc.sync.dma_start(
                    out=out_h.ap()[lo:hi, :, D : 2 * D], in_=acc3[lo:hi]
                )
    nc.compile()
    return nc


def kernel(x_locs, x_clusters, edge_src, edge_dst):
    global LAST_RESULTS, LAST_NC
    from concourse.bass_utils import run_bass_kernel_spmd

    in_maps, order, M, offs, TOT, _xc = _host_prep(
        x_locs, x_clusters, edge_src, edge_dst
    )
    R = len(M)
    nc = _build_program(R, M, offs, TOT)
    LAST_NC = nc
    try:
        res = run_bass_kernel_spmd(nc, in_maps, list(range(N_CORES)))
    except Exception:
        # transient NRT/tunnel faults (e.g. NRT_EXEC_UNIT_UNRECOVERABLE from
        # a prior session) clear on re-execution; retry once
        res = run_bass_kernel_spmd(nc, in_maps, list(range(N_CORES)))
    LAST_RESULTS = res

    full = np.empty((N_CLUSTERS, 2 * D), dtype=np.float32)
    for k in range(N_CORES):
        o = np.asarray(res.results[k]["out"])  # [P, CHUNKS, 2D]
        o = o.transpose(1, 0, 2).reshape(CPC, 2 * D)  # indexed by sorted rank
        full[order[k]] = o
    return full



# revision 6
# speedup vs baseline: 2.0879x; 2.0879x over previous
"""Trainium2 Bass kernel for nn_Loc2Cluster (GNN message passing, segment-max).

Computation: agg[c] = elementwise-max over locs with edge to cluster c of
x_locs[loc]; empty clusters -> 0; output = concat([x_clusters, agg], -1).

Strategy (cluster-sharded, zero collectives, bf16 streams):
  - Core k owns clusters [4096k, 4096(k+1)) after a global count-desc sort
    dealt round-robin across cores (balances per-core round sizes to +-1).
  - Host routes each edge's loc row (pre-converted to bf16; rel tol is
    2e-2, bf16 rounding is ~2^-9) to the core owning its dst cluster.
  - Rows are laid out in "rounds": round r holds the r-th edge row of every
    cluster with count > r, in sorted order, so each round's segment-max is
    a plain elementwise tensor_max over a prefix of cluster slots.
  - Big rounds (m_r > 128) are partition-major [128, X*256] blocks padded
    to whole 128-cluster chunks with -1e30 (max-neutral): one DMA + one max.
  - Tail rounds (m_r <= 128) all touch only chunk 0; consecutive ones are
    merged into grouped DMA blocks [T*M, 256] -> st[0:M, :T*256] so the
    instruction count in the tiny-round tail stays low.
  - Round 0 is DMA'd straight into the accumulator (zero rows for empty
    clusters match the reference's 0-fill).
  - Output chunks are written back as soon as no later round touches them,
    overlapping the agg writeback with the remaining stream DMAs.
  - x_clusters never touches the device: the concat left half is assembled
    on the host at full fp32 precision during unsharding.
"""

import sys

import numpy as np

if "/opt/trn_rl_repo" not in sys.path:
    sys.path.insert(0, "/opt/trn_rl_repo")

import ml_dtypes

BF16 = ml_dtypes.bfloat16

N_LOCS = 262144
N_CLUSTERS = 32768
D = 256
N_CORES = 8
CPC = N_CLUSTERS // N_CORES  # 4096 clusters per core
P = 128
CHUNKS = CPC // P  # 32 chunks of 128 clusters
NEG = np.float32(-1e30)

LAST_RESULTS = None  # BassKernelResults of the most recent run (for profiling)
LAST_NC = None  # compiled Bass module of the most recent run (for TimelineSim)


def _make_schedule(m):
    """Turn per-core round sizes m[r] into a device schedule.

    Returns (items, offs_of_round, TOT):
      items: list of ("A", off, X, r) for big rounds (X full/padded chunks)
             or ("G", off, Mstar, [(r, m_r), ...]) for merged tail groups.
      offs_of_round[r]: row offset of round r's block in the stream.
    """
    R = len(m)
    items = []
    offs_of_round = np.zeros(R, dtype=np.int64)
    pos = 0
    r = 0
    while r < R:
        if r == 0 or m[r] > P:
            X = int(-(-m[r] // P)) if r > 0 else CHUNKS
            offs_of_round[r] = pos
            items.append(("A", pos, X, r))
            pos += X * P
            r += 1
        else:
            # greedy tail group: consecutive rounds, new group when the
            # first round's size dwarfs the current one (pad cost) or the
            # group would outgrow the stage tile width
            Mstar = int(m[r])
            members = []
            while (
                r < R
                and m[r] <= P
                and m[r] * 4 >= Mstar
                and len(members) < CHUNKS
            ):
                members.append((r, int(m[r])))
                offs_of_round[r] = pos + len(members[:-1]) * Mstar
                r += 1
            items.append(("G", pos, Mstar, members))
            pos += Mstar * len(members)
    return items, offs_of_round, pos


def _host_prep(x_locs, x_clusters, edge_src, edge_dst):
    """Build per-core bf16 round-major row streams."""
    x_locs = np.asarray(x_locs, dtype=np.float32)
    x_bf = x_locs.astype(BF16)
    src = np.asarray(edge_src).astype(np.int64)
    dst = np.asarray(edge_dst).astype(np.int64)
    n_edges = dst.shape[0]

    counts = np.bincount(dst, minlength=N_CLUSTERS)  # [32768]

    # Global order by count desc, dealt round-robin across cores: cluster
    # with global rank g goes to core g%8 at local rank g//8, so the shared
    # (SPMD) round schedule has at most 1 pad row per round per core.
    gorder = np.argsort(-counts, kind="stable")
    grank = np.empty_like(gorder)
    grank[gorder] = np.arange(N_CLUSTERS)
    order = np.ascontiguousarray(gorder.reshape(CPC, N_CORES).T)  # [8, CPC]

    # occurrence index of each edge within its dst cluster
    by_dst = np.argsort(dst, kind="stable")
    group_start = np.zeros(N_CLUSTERS, dtype=np.int64)
    np.cumsum(counts[:-1], out=group_start[1:])
    occ = np.empty(n_edges, dtype=np.int64)
    occ[by_dst] = np.arange(n_edges, dtype=np.int64) - group_start[dst[by_dst]]

    g_of = grank[dst]
    core_of = g_of % N_CORES
    rank_of = g_of // N_CORES  # local rank s, count-sorted desc

    R = max(int(counts.max()), 1)
    counts_sorted = counts[gorder]
    m_r_g = (counts_sorted[None, :] > np.arange(R)[:, None]).sum(axis=1)
    m = (m_r_g + N_CORES - 1) // N_CORES  # shared per-core round size
    m[0] = CPC  # round 0 covers every slot (zeros for empty clusters)

    items, offs_of_round, TOT = _make_schedule(m)

    # per-round layout params for the slot formula
    X_of = np.ones(R, dtype=np.int64)  # chunks per round (A rounds)
    is_A = np.zeros(R, dtype=bool)
    for it in items:
        if it[0] == "A":
            X_of[it[3]] = it[2]
            is_A[it[3]] = True

    ro = offs_of_round[occ]
    slot = np.where(
        is_A[occ],
        ro + (rank_of % P) * X_of[occ] + rank_of // P,
        ro + rank_of,  # tail rounds: single chunk, p = rank
    )

    slot_src = np.full((N_CORES, TOT), -1, dtype=np.int64)
    slot_src[core_of, slot] = src

    in_maps = []
    for k in range(N_CORES):
        ss = slot_src[k]
        stream = x_bf[np.maximum(ss, 0)]  # [TOT, 256] bf16
        pad = ss < 0
        p0 = np.flatnonzero(pad[:CPC])
        if p0.size:
            stream[p0] = 0.0  # empty clusters -> 0
        pr = np.flatnonzero(pad[CPC:]) + CPC
        if pr.size:
            stream[pr] = NEG  # later-round pads are max-neutral
        in_maps.append({"rows": np.ascontiguousarray(stream)})

    return in_maps, order, items, TOT


def _build_program(items, TOT, R, bufs=8):
    from concourse import bacc, mybir
    from concourse._compat import axon_active
    from concourse.tile import TileContext

    nc = bacc.Bacc(
        "TRN2",
        target_bir_lowering=False,
        debug=not axon_active(),
        num_devices=N_CORES,
    )
    rows_h = nc.dram_tensor("rows", [TOT, D], mybir.dt.bfloat16, kind="ExternalInput")
    out_h = nc.dram_tensor(
        "out", [P, CHUNKS, D], mybir.dt.bfloat16, kind="ExternalOutput"
    )

    # chunks touched per schedule item (non-increasing)
    touches = [it[2] if it[0] == "A" else 1 for it in items]

    with TileContext(nc) as tc:
        with (
            tc.tile_pool(name="accp", bufs=1) as accp,
            tc.tile_pool(name="stagep", bufs=bufs) as stagep,
        ):
            acc = accp.tile([P, CHUNKS * D], mybir.dt.bfloat16)
            acc3 = acc[:].rearrange("p (x f) -> p x f", f=D)
            pending_hi = CHUNKS

            def flush(next_touch):
                nonlocal pending_hi
                if next_touch < pending_hi:
                    # scalar-queue issue: a DMA's sem wait holds its SEQ, so
                    # putting writebacks on SP would stall stream-DMA issue
                    nc.scalar.dma_start(
                        out=out_h.ap()[:, next_touch:pending_hi, :],
                        in_=acc3[:, next_touch:pending_hi, :],
                    )
                    pending_hi = next_touch

            for i, it in enumerate(items):
                if it[0] == "A":
                    _, off, X, r = it
                    blk = rows_h.ap()[off : off + X * P].rearrange(
                        "(p x) f -> p (x f)", p=P
                    )
                    w = X * D
                    if r == 0:
                        nc.sync.dma_start(out=acc[:, :w], in_=blk)
                    else:
                        st = stagep.tile([P, CHUNKS * D], mybir.dt.bfloat16, tag="st")
                        nc.sync.dma_start(out=st[:, :w], in_=blk)
                        nc.vector.tensor_max(
                            out=acc[:, :w], in0=acc[:, :w], in1=st[:, :w]
                        )
                else:
                    _, off, Mstar, members = it
                    T = len(members)
                    st = stagep.tile([P, CHUNKS * D], mybir.dt.bfloat16, tag="st")
                    blk = rows_h.ap()[off : off + T * Mstar].rearrange(
                        "(t p) f -> p t f", p=Mstar
                    )
                    st3 = st[0:Mstar, : T * D].rearrange("p (t f) -> p t f", f=D)
                    nc.sync.dma_start(out=st3, in_=blk)
                    for t, (r, mr) in enumerate(members):
                        nc.vector.tensor_max(
                            out=acc[0:mr, 0:D],
                            in0=acc[0:mr, 0:D],
                            in1=st[0:mr, t * D : (t + 1) * D],
                        )
                flush(touches[i + 1] if i + 1 < len(items) else 0)
    nc.compile()
    return nc


def kernel(x_locs, x_clusters, edge_src, edge_dst):
    global LAST_RESULTS, LAST_NC
    from concourse.bass_utils import run_bass_kernel_spmd

    x_clusters = np.ascontiguousarray(np.asarray(x_clusters, dtype=np.float32))
    in_maps, order, items, TOT = _host_prep(x_locs, x_clusters, edge_src, edge_dst)
    nc = _build_program(items, TOT, len(items))
    LAST_NC = nc
    try:
        res = run_bass_kernel_spmd(nc, in_maps, list(range(N_CORES)))
    except Exception:
        # transient NRT/tunnel faults clear on re-execution; retry once
        res = run_bass_kernel_spmd(nc, in_maps, list(range(N_CORES)))
    LAST_RESULTS = res

    full = np.empty((N_CLUSTERS, 2 * D), dtype=np.float32)
    full[:, :D] = x_clusters
    for k in range(N_CORES):
        o = np.asarray(res.results[k]["out"])  # [P, CHUNKS, D] bf16
        o = o.transpose(1, 0, 2).reshape(CPC, D)  # indexed by sorted rank
        full[order[k], D:] = o.astype(np.float32)
    return full


# revision 9
# speedup vs baseline: 2.0884x; 1.0002x over previous
"""Trainium2 Bass kernel for nn_Loc2Cluster (GNN message passing, segment-max).

Computation: agg[c] = elementwise-max over locs with edge to cluster c of
x_locs[loc]; empty clusters -> 0; output = concat([x_clusters, agg], -1).

Strategy (cluster-sharded, zero collectives, bf16 streams):
  - Core k owns clusters [4096k, 4096(k+1)) after a global count-desc sort
    dealt round-robin across cores (balances per-core round sizes to +-1).
  - Host routes each edge's loc row (pre-converted to bf16; rel tol is
    2e-2, bf16 rounding is ~2^-9) to the core owning its dst cluster.
  - Rows are laid out in "rounds": round r holds the r-th edge row of every
    cluster with count > r, in sorted order, so each round's segment-max is
    a plain elementwise tensor_max over a prefix of cluster slots.
  - Big rounds (m_r > 128) are partition-major [128, X*256] blocks padded
    to whole 128-cluster chunks with -1e30 (max-neutral): one DMA + one max.
  - Tail rounds (m_r <= 128) all touch only chunk 0; consecutive ones are
    merged into grouped DMA blocks [T*M, 256] -> st[0:M, :T*256] so the
    instruction count in the tiny-round tail stays low.
  - Round 0 is DMA'd straight into the accumulator (zero rows for empty
    clusters match the reference's 0-fill).
  - Output chunks are written back as soon as no later round touches them,
    overlapping the agg writeback with the remaining stream DMAs.
  - x_clusters never touches the device: the concat left half is assembled
    on the host at full fp32 precision during unsharding.
"""

import sys

import numpy as np

if "/opt/trn_rl_repo" not in sys.path:
    sys.path.insert(0, "/opt/trn_rl_repo")

import ml_dtypes

BF16 = ml_dtypes.bfloat16

N_LOCS = 262144
N_CLUSTERS = 32768
D = 256
N_CORES = 8
CPC = N_CLUSTERS // N_CORES  # 4096 clusters per core
P = 128
CHUNKS = CPC // P  # 32 chunks of 128 clusters
NEG = np.float32(-1e30)

LAST_RESULTS = None  # BassKernelResults of the most recent run (for profiling)
LAST_NC = None  # compiled Bass module of the most recent run (for TimelineSim)


def _make_schedule(m):
    """Turn per-core round sizes m[r] into a device schedule.

    Returns (items, offs_of_round, TOT):
      items: list of ("A", off, X, r) for big rounds (X full/padded chunks)
             or ("G", off, Mstar, [(r, m_r), ...]) for merged tail groups.
      offs_of_round[r]: row offset of round r's block in the stream.
    """
    R = len(m)
    items = []
    offs_of_round = np.zeros(R, dtype=np.int64)
    pos = 0
    r = 0
    while r < R:
        if r == 0 or m[r] > P:
            X = int(-(-m[r] // P)) if r > 0 else CHUNKS
            offs_of_round[r] = pos
            items.append(("A", pos, X, r))
            pos += X * P
            r += 1
        else:
            # greedy tail group: consecutive rounds, new group when the
            # first round's size dwarfs the current one (pad cost) or the
            # group would outgrow the stage tile width
            Mstar = int(m[r])
            members = []
            while (
                r < R
                and m[r] <= P
                and m[r] * 4 >= Mstar
                and len(members) < CHUNKS
            ):
                members.append((r, int(m[r])))
                offs_of_round[r] = pos + len(members[:-1]) * Mstar
                r += 1
            items.append(("G", pos, Mstar, members))
            pos += Mstar * len(members)
    return items, offs_of_round, pos


def _host_prep(x_locs, x_clusters, edge_src, edge_dst):
    """Build per-core bf16 round-major row streams."""
    x_locs = np.asarray(x_locs, dtype=np.float32)
    x_bf = x_locs.astype(BF16)
    src = np.asarray(edge_src).astype(np.int64)
    dst = np.asarray(edge_dst).astype(np.int64)
    n_edges = dst.shape[0]

    counts = np.bincount(dst, minlength=N_CLUSTERS)  # [32768]

    # Global order by count desc, dealt round-robin across cores: cluster
    # with global rank g goes to core g%8 at local rank g//8, so the shared
    # (SPMD) round schedule has at most 1 pad row per round per core.
    gorder = np.argsort(-counts, kind="stable")
    grank = np.empty_like(gorder)
    grank[gorder] = np.arange(N_CLUSTERS)
    order = np.ascontiguousarray(gorder.reshape(CPC, N_CORES).T)  # [8, CPC]

    # occurrence index of each edge within its dst cluster
    by_dst = np.argsort(dst, kind="stable")
    group_start = np.zeros(N_CLUSTERS, dtype=np.int64)
    np.cumsum(counts[:-1], out=group_start[1:])
    occ = np.empty(n_edges, dtype=np.int64)
    occ[by_dst] = np.arange(n_edges, dtype=np.int64) - group_start[dst[by_dst]]

    g_of = grank[dst]
    core_of = g_of % N_CORES
    rank_of = g_of // N_CORES  # local rank s, count-sorted desc

    R = max(int(counts.max()), 1)
    counts_sorted = counts[gorder]
    m_r_g = (counts_sorted[None, :] > np.arange(R)[:, None]).sum(axis=1)
    m = (m_r_g + N_CORES - 1) // N_CORES  # shared per-core round size
    m[0] = CPC  # round 0 covers every slot (zeros for empty clusters)

    items, offs_of_round, TOT = _make_schedule(m)

    # per-round layout params for the slot formula
    X_of = np.ones(R, dtype=np.int64)  # chunks per round (A rounds)
    is_A = np.zeros(R, dtype=bool)
    for it in items:
        if it[0] == "A":
            X_of[it[3]] = it[2]
            is_A[it[3]] = True

    ro = offs_of_round[occ]
    slot = np.where(
        is_A[occ],
        ro + (rank_of % P) * X_of[occ] + rank_of // P,
        ro + rank_of,  # tail rounds: single chunk, p = rank
    )

    slot_src = np.full((N_CORES, TOT), -1, dtype=np.int64)
    slot_src[core_of, slot] = src

    in_maps = []
    for k in range(N_CORES):
        ss = slot_src[k]
        stream = x_bf[np.maximum(ss, 0)]  # [TOT, 256] bf16
        pad = ss < 0
        p0 = np.flatnonzero(pad[:CPC])
        if p0.size:
            stream[p0] = 0.0  # empty clusters -> 0
        pr = np.flatnonzero(pad[CPC:]) + CPC
        if pr.size:
            stream[pr] = NEG  # later-round pads are max-neutral
        in_maps.append({"rows": np.ascontiguousarray(stream)})

    return in_maps, order, items, TOT


def _build_program(items, TOT, R, bufs=8):
    from concourse import bacc, mybir
    from concourse._compat import axon_active
    from concourse.tile import TileContext

    nc = bacc.Bacc(
        "TRN2",
        target_bir_lowering=False,
        debug=not axon_active(),
        num_devices=N_CORES,
    )
    rows_h = nc.dram_tensor("rows", [TOT, D], mybir.dt.bfloat16, kind="ExternalInput")
    out_h = nc.dram_tensor(
        "out", [P, CHUNKS, D], mybir.dt.bfloat16, kind="ExternalOutput"
    )

    # Execution order: round 0, big rounds descending, then tail groups,
    # with the smallest A-round LAST — its single small tensor_max is the
    # shortest possible post-DMA critical path before the final flush,
    # while the tail groups' max chains hide under the last A-DMAs.
    a_items = [it for it in items if it[0] == "A"]
    g_items = [it for it in items if it[0] == "G"]
    if len(a_items) > 1 and g_items:
        items = a_items[:-1] + g_items + [a_items[-1]]

    # chunks touched per schedule item
    touches = [it[2] if it[0] == "A" else 1 for it in items]
    # chunks still touched by items after position i
    remain = [0] * len(items)
    acc_t = 0
    for i in range(len(items) - 1, -1, -1):
        remain[i] = acc_t
        acc_t = max(acc_t, touches[i])

    with TileContext(nc) as tc:
        with (
            tc.tile_pool(name="accp", bufs=1) as accp,
            tc.tile_pool(name="stagep", bufs=bufs) as stagep,
        ):
            acc = accp.tile([P, CHUNKS * D], mybir.dt.bfloat16)
            acc3 = acc[:].rearrange("p (x f) -> p x f", f=D)
            pending_hi = CHUNKS

            def flush(next_touch, final=False):
                nonlocal pending_hi
                # merge tiny writebacks: only flush >=3 chunks unless final
                if next_touch < pending_hi and (
                    final or pending_hi - next_touch >= 3
                ):
                    # scalar-queue issue: a DMA's sem wait holds its SEQ, so
                    # putting writebacks on SP would stall stream-DMA issue
                    nc.scalar.dma_start(
                        out=out_h.ap()[:, next_touch:pending_hi, :],
                        in_=acc3[:, next_touch:pending_hi, :],
                    )
                    pending_hi = next_touch

            for i, it in enumerate(items):
                if it[0] == "A":
                    _, off, X, r = it
                    blk = rows_h.ap()[off : off + X * P].rearrange(
                        "(p x) f -> p (x f)", p=P
                    )
                    w = X * D
                    if r == 0:
                        nc.sync.dma_start(out=acc[:, :w], in_=blk)
                    else:
                        st = stagep.tile([P, CHUNKS * D], mybir.dt.bfloat16, tag="st")
                        nc.sync.dma_start(out=st[:, :w], in_=blk)
                        nc.vector.tensor_max(
                            out=acc[:, :w], in0=acc[:, :w], in1=st[:, :w]
                        )
                else:
                    _, off, Mstar, members = it
                    T = len(members)
                    st = stagep.tile([P, CHUNKS * D], mybir.dt.bfloat16, tag="st")
                    blk = rows_h.ap()[off : off + T * Mstar].rearrange(
                        "(t p) f -> p t f", p=Mstar
                    )
                    st3 = st[0:Mstar, : T * D].rearrange("p (t f) -> p t f", f=D)
                    nc.sync.dma_start(out=st3, in_=blk)
                    for t, (r, mr) in enumerate(members):
                        nc.vector.tensor_max(
                            out=acc[0:mr, 0:D],
                            in0=acc[0:mr, 0:D],
                            in1=st[0:mr, t * D : (t + 1) * D],
                        )
                flush(remain[i], final=(i == len(items) - 1))
    nc.compile()
    return nc


def kernel(x_locs, x_clusters, edge_src, edge_dst):
    global LAST_RESULTS, LAST_NC
    from concourse.bass_utils import run_bass_kernel_spmd

    x_clusters = np.ascontiguousarray(np.asarray(x_clusters, dtype=np.float32))
    in_maps, order, items, TOT = _host_prep(x_locs, x_clusters, edge_src, edge_dst)
    nc = _build_program(items, TOT, len(items))
    LAST_NC = nc
    try:
        res = run_bass_kernel_spmd(nc, in_maps, list(range(N_CORES)))
    except Exception:
        # transient NRT/tunnel faults clear on re-execution; retry once
        res = run_bass_kernel_spmd(nc, in_maps, list(range(N_CORES)))
    LAST_RESULTS = res

    full = np.empty((N_CLUSTERS, 2 * D), dtype=np.float32)
    full[:, :D] = x_clusters
    for k in range(N_CORES):
        o = np.asarray(res.results[k]["out"])  # [P, CHUNKS, D] bf16
        o = o.transpose(1, 0, 2).reshape(CPC, D)  # indexed by sorted rank
        full[order[k], D:] = o.astype(np.float32)
    return full


# revision 11
# speedup vs baseline: 2.1562x; 1.0325x over previous
"""Trainium2 Bass kernel for nn_Loc2Cluster (GNN message passing, segment-max).

Computation: agg[c] = elementwise-max over locs with edge to cluster c of
x_locs[loc]; empty clusters -> 0; output = concat([x_clusters, agg], -1).

Strategy (cluster-sharded, zero collectives, bf16 streams):
  - Core k owns clusters [4096k, 4096(k+1)) after a global count-desc sort
    dealt round-robin across cores (balances per-core round sizes to +-1).
  - Host routes each edge's loc row (pre-converted to bf16; rel tol is
    2e-2, bf16 rounding is ~2^-9) to the core owning its dst cluster.
  - Rows are laid out in "rounds": round r holds the r-th edge row of every
    cluster with count > r, in local-rank order (chunk-major), so each
    round's segment-max is elementwise tensor_max over a prefix of slots.
  - Big rounds (m_r > 128) stream exactly m_r rows: full chunks as one
    chunk-major DMA + the partial last chunk as a small second DMA, each
    followed by its own tensor_max. No pad bytes.
  - Tail rounds (m_r <= 128) all touch only chunk 0; consecutive ones are
    merged into grouped DMA blocks (padded to the group max with -1e30,
    which is max-neutral) and are scheduled right after round 0 so their
    serial max chains hide under the big-round stream DMAs.
  - The last big round is split so the final flush waits only on one small
    [128, 256] tensor_max.
  - Round 0 is DMA'd straight into the accumulator (zero rows for empty
    clusters match the reference's 0-fill).
  - Output chunks are written back (scalar-engine queue) as soon as no
    later item touches them, overlapping writeback with the stream.
  - x_clusters never touches the device: the concat left half is assembled
    on the host at full fp32 precision during unsharding.
"""

import sys

import numpy as np

if "/opt/trn_rl_repo" not in sys.path:
    sys.path.insert(0, "/opt/trn_rl_repo")

import ml_dtypes

BF16 = ml_dtypes.bfloat16

N_LOCS = 262144
N_CLUSTERS = 32768
D = 256
N_CORES = 8
CPC = N_CLUSTERS // N_CORES  # 4096 clusters per core
P = 128
CHUNKS = CPC // P  # 32 chunks of 128 clusters
NEG = np.float32(-1e30)

LAST_RESULTS = None  # BassKernelResults of the most recent run (for profiling)
LAST_NC = None  # compiled Bass module of the most recent run (for TimelineSim)


def _make_schedule(m):
    """Turn per-core round sizes m[r] into an execution schedule.

    Returns (items, offs_of_round, TOT). items are in execution order:
      ("R0",)                      round 0, CPC rows into the accumulator
      ("A", off, mr)               big round, exact mr rows (q full chunks
                                   chunk-major + rem-row partial chunk)
      ("AB", off, q, rem)          partial-chunk piece of the split last round
      ("AA", off, q)               full-chunk piece of the split last round
      ("G", off, Mstar, members)   merged tail rounds, members=[(r, m_r)]
    Stream layout (HBM row offsets) is in round order; execution order is
    [R0, tail groups..., big rounds desc, last round's B then A piece].
    """
    R = len(m)
    offs_of_round = np.zeros(R, dtype=np.int64)
    a_items = []
    g_items = []
    pos = 0
    r = 1
    offs_of_round[0] = 0
    pos = CPC
    while r < R:
        if m[r] > P:
            offs_of_round[r] = pos
            a_items.append(("A", int(pos), int(m[r])))
            pos += int(m[r])
            r += 1
        else:
            # greedy tail group of consecutive small rounds; new group when
            # padding to the group max would get wasteful
            Mstar = int(m[r])
            members = []
            gpos = pos
            while r < R and m[r] <= P and m[r] * 4 >= Mstar and len(members) < CHUNKS:
                members.append((r, int(m[r])))
                offs_of_round[r] = gpos
                gpos += Mstar
                r += 1
            g_items.append(("G", int(pos), Mstar, members))
            pos = gpos
    items = [("R0",)] + g_items + a_items
    # split the last big round so the final flush gates on one small max
    if a_items:
        kind, off, mr = a_items[-1]
        q, rem = divmod(mr, P)
        if q > 0:
            items = items[:-1]
            if rem > 0:
                items.append(("AB", off + q * P, q, rem))
            items.append(("AA", off, q))
    return items, offs_of_round, int(pos)


def _host_prep(x_locs, x_clusters, edge_src, edge_dst):
    """Build per-core bf16 round-major row streams."""
    x_locs = np.asarray(x_locs, dtype=np.float32)
    x_bf = x_locs.astype(BF16)
    src = np.asarray(edge_src).astype(np.int64)
    dst = np.asarray(edge_dst).astype(np.int64)
    n_edges = dst.shape[0]

    counts = np.bincount(dst, minlength=N_CLUSTERS)  # [32768]

    gorder = np.argsort(-counts, kind="stable")
    grank = np.empty_like(gorder)
    grank[gorder] = np.arange(N_CLUSTERS)
    order = np.ascontiguousarray(gorder.reshape(CPC, N_CORES).T)  # [8, CPC]

    # occurrence index of each edge within its dst cluster
    by_dst = np.argsort(dst, kind="stable")
    group_start = np.zeros(N_CLUSTERS, dtype=np.int64)
    np.cumsum(counts[:-1], out=group_start[1:])
    occ = np.empty(n_edges, dtype=np.int64)
    occ[by_dst] = np.arange(n_edges, dtype=np.int64) - group_start[dst[by_dst]]

    g_of = grank[dst]
    core_of = g_of % N_CORES
    rank_of = g_of // N_CORES  # local rank s, count-sorted desc

    R = max(int(counts.max()), 1)
    counts_sorted = counts[gorder]
    m_r_g = (counts_sorted[None, :] > np.arange(R)[:, None]).sum(axis=1)
    m = (m_r_g + N_CORES - 1) // N_CORES  # shared per-core round size
    m[0] = CPC  # round 0 covers every slot (zeros for empty clusters)

    items, offs_of_round, TOT = _make_schedule(m)

    # chunk-major blocks everywhere: slot within a round block == local rank
    slot = offs_of_round[occ] + rank_of

    slot_src = np.full((N_CORES, TOT), -1, dtype=np.int64)
    slot_src[core_of, slot] = src

    in_maps = []
    for k in range(N_CORES):
        ss = slot_src[k]
        stream = x_bf[np.maximum(ss, 0)]  # [TOT, 256] bf16
        pad = ss < 0
        p0 = np.flatnonzero(pad[:CPC])
        if p0.size:
            stream[p0] = 0.0  # empty clusters -> 0
        pr = np.flatnonzero(pad[CPC:]) + CPC
        if pr.size:
            stream[pr] = NEG  # pads (group + straggler) are max-neutral
        in_maps.append({"rows": np.ascontiguousarray(stream)})

    return in_maps, order, items, TOT


def _build_program(items, TOT, bufs=6):
    from concourse import bacc, mybir
    from concourse._compat import axon_active
    from concourse.tile import TileContext

    nc = bacc.Bacc(
        "TRN2",
        target_bir_lowering=False,
        debug=not axon_active(),
        num_devices=N_CORES,
    )
    rows_h = nc.dram_tensor("rows", [TOT, D], mybir.dt.bfloat16, kind="ExternalInput")
    out_h = nc.dram_tensor(
        "out", [P, CHUNKS, D], mybir.dt.bfloat16, kind="ExternalOutput"
    )

    # per-chunk last-toucher item index, for early writeback
    def touched(it):
        if it[0] == "R0":
            return range(CHUNKS)
        if it[0] == "A":
            q, rem = divmod(it[2], P)
            return range(q + (1 if rem else 0))
        if it[0] == "AB":
            return range(it[2], it[2] + 1)
        if it[0] == "AA":
            return range(it[2])
        return range(1)  # G: chunk 0 only

    last_touch = [0] * CHUNKS
    for i, it in enumerate(items):
        for c in touched(it):
            last_touch[c] = i

    with TileContext(nc) as tc:
        with (
            tc.tile_pool(name="accp", bufs=1) as accp,
            tc.tile_pool(name="stagep", bufs=bufs) as stagep,
        ):
            acc = accp.tile([P, CHUNKS * D], mybir.dt.bfloat16)
            acc3 = acc[:].rearrange("p (x f) -> p x f", f=D)
            pending_hi = CHUNKS

            def flush(i):
                nonlocal pending_hi
                lo = pending_hi
                while lo > 0 and last_touch[lo - 1] <= i:
                    lo -= 1
                final = i == len(items) - 1
                # merge tiny writebacks: only flush >=3 chunks unless final
                if lo < pending_hi and (final or pending_hi - lo >= 3):
                    # scalar-queue issue: a DMA's sem wait holds its SEQ, so
                    # putting writebacks on SP would stall stream-DMA issue
                    nc.scalar.dma_start(
                        out=out_h.ap()[:, lo:pending_hi, :],
                        in_=acc3[:, lo:pending_hi, :],
                    )
                    pending_hi = lo

            def dma_a_piece(dst, off, q):
                blk = rows_h.ap()[off : off + q * P].rearrange(
                    "(x p) f -> p x f", p=P
                )
                dst3 = dst[:, : q * D].rearrange("p (x f) -> p x f", f=D)
                nc.sync.dma_start(out=dst3, in_=blk)

            def dma_b_piece(dst, off, q, rem):
                nc.sync.dma_start(
                    out=dst[0:rem, q * D : (q + 1) * D],
                    in_=rows_h.ap()[off : off + rem],
                )

            for i, it in enumerate(items):
                if it[0] == "R0":
                    dma_a_piece(acc, 0, CHUNKS)
                elif it[0] == "A":
                    _, off, mr = it
                    q, rem = divmod(mr, P)
                    st = stagep.tile([P, CHUNKS * D], mybir.dt.bfloat16, tag="st")
                    if q > 0:
                        dma_a_piece(st, off, q)
                    if rem > 0:
                        dma_b_piece(st, off + q * P, q, rem)
                    if q > 0:
                        w = q * D
                        nc.vector.tensor_max(
                            out=acc[:, :w], in0=acc[:, :w], in1=st[:, :w]
                        )
                    if rem > 0:
                        sl = slice(q * D, (q + 1) * D)
                        nc.vector.tensor_max(
                            out=acc[0:rem, sl], in0=acc[0:rem, sl], in1=st[0:rem, sl]
                        )
                elif it[0] == "AB":
                    _, off, q, rem = it
                    st = stagep.tile([P, CHUNKS * D], mybir.dt.bfloat16, tag="st")
                    dma_b_piece(st, off, q, rem)
                    sl = slice(q * D, (q + 1) * D)
                    nc.vector.tensor_max(
                        out=acc[0:rem, sl], in0=acc[0:rem, sl], in1=st[0:rem, sl]
                    )
                elif it[0] == "AA":
                    _, off, q = it
                    st = stagep.tile([P, CHUNKS * D], mybir.dt.bfloat16, tag="st")
                    dma_a_piece(st, off, q)
                    w = q * D
                    nc.vector.tensor_max(
                        out=acc[:, :w], in0=acc[:, :w], in1=st[:, :w]
                    )
                else:  # G
                    _, off, Mstar, members = it
                    T = len(members)
                    st = stagep.tile([P, CHUNKS * D], mybir.dt.bfloat16, tag="st")
                    blk = rows_h.ap()[off : off + T * Mstar].rearrange(
                        "(t p) f -> p t f", p=Mstar
                    )
                    st3 = st[0:Mstar, : T * D].rearrange("p (t f) -> p t f", f=D)
                    nc.sync.dma_start(out=st3, in_=blk)
                    for t, (r, mr) in enumerate(members):
                        nc.vector.tensor_max(
                            out=acc[0:mr, 0:D],
                            in0=acc[0:mr, 0:D],
                            in1=st[0:mr, t * D : (t + 1) * D],
                        )
                flush(i)
    nc.compile()
    return nc


def kernel(x_locs, x_clusters, edge_src, edge_dst):
    global LAST_RESULTS, LAST_NC
    from concourse.bass_utils import run_bass_kernel_spmd

    x_clusters = np.ascontiguousarray(np.asarray(x_clusters, dtype=np.float32))
    in_maps, order, items, TOT = _host_prep(x_locs, x_clusters, edge_src, edge_dst)
    nc = _build_program(items, TOT)
    LAST_NC = nc
    try:
        res = run_bass_kernel_spmd(nc, in_maps, list(range(N_CORES)))
    except Exception:
        # transient NRT/tunnel faults clear on re-execution; retry once
        res = run_bass_kernel_spmd(nc, in_maps, list(range(N_CORES)))
    LAST_RESULTS = res

    full = np.empty((N_CLUSTERS, 2 * D), dtype=np.float32)
    full[:, :D] = x_clusters
    for k in range(N_CORES):
        o = np.asarray(res.results[k]["out"])  # [P, CHUNKS, D] bf16
        o = o.transpose(1, 0, 2).reshape(CPC, D)  # indexed by sorted rank
        full[order[k], D:] = o.astype(np.float32)
    return full


# revision 13
# speedup vs baseline: 2.1642x; 1.0037x over previous
"""Trainium2 Bass kernel for nn_Loc2Cluster (GNN message passing, segment-max).

Computation: agg[c] = elementwise-max over locs with edge to cluster c of
x_locs[loc]; empty clusters -> 0; output = concat([x_clusters, agg], -1).

Strategy (cluster-sharded, zero collectives, bf16 streams):
  - Core k owns clusters [4096k, 4096(k+1)) after a global count-desc sort
    dealt round-robin across cores (balances per-core round sizes to +-1).
  - Host routes each edge's loc row (pre-converted to bf16; rel tol is
    2e-2, bf16 rounding is ~2^-9) to the core owning its dst cluster.
  - Rows are laid out in "rounds": round r holds the r-th edge row of every
    cluster with count > r, in local-rank order (chunk-major), so each
    round's segment-max is elementwise tensor_max over a prefix of slots.
  - Big rounds (m_r > 128) stream exactly m_r rows: full chunks as one
    chunk-major DMA + the partial last chunk as a small second DMA, each
    followed by its own tensor_max. No pad bytes.
  - Tail rounds (m_r <= 128) all touch only chunk 0; consecutive ones are
    merged into grouped DMA blocks (padded to the group max with -1e30,
    which is max-neutral) and are scheduled right after round 0 so their
    serial max chains hide under the big-round stream DMAs.
  - The last big round is split so the final flush waits only on one small
    [128, 256] tensor_max.
  - Round 0 is DMA'd straight into the accumulator (zero rows for empty
    clusters match the reference's 0-fill).
  - Output chunks are written back (scalar-engine queue) as soon as no
    later item touches them, overlapping writeback with the stream.
  - x_clusters never touches the device: the concat left half is assembled
    on the host at full fp32 precision during unsharding.
"""

import sys

import numpy as np

if "/opt/trn_rl_repo" not in sys.path:
    sys.path.insert(0, "/opt/trn_rl_repo")

import ml_dtypes

BF16 = ml_dtypes.bfloat16

N_LOCS = 262144
N_CLUSTERS = 32768
D = 256
N_CORES = 8
CPC = N_CLUSTERS // N_CORES  # 4096 clusters per core
P = 128
CHUNKS = CPC // P  # 32 chunks of 128 clusters
NEG = np.float32(-1e30)

LAST_RESULTS = None  # BassKernelResults of the most recent run (for profiling)
LAST_NC = None  # compiled Bass module of the most recent run (for TimelineSim)


def _make_schedule(m):
    """Turn per-core round sizes m[r] into an execution schedule.

    Returns (items, offs_of_round, TOT). items are in execution order:
      ("R0",)                      round 0, CPC rows into the accumulator
      ("A", off, mr)               big round, exact mr rows (q full chunks
                                   chunk-major + rem-row partial chunk)
      ("AB", off, q, rem)          partial-chunk piece of the split last round
      ("AA", off, q)               full-chunk piece of the split last round
      ("G", off, Mstar, members)   merged tail rounds, members=[(r, m_r)]
    Stream layout (HBM row offsets) is in round order; execution order is
    [R0, tail groups..., big rounds desc, last round's B then A piece].
    """
    R = len(m)
    offs_of_round = np.zeros(R, dtype=np.int64)
    a_items = []
    g_items = []
    pos = 0
    r = 1
    offs_of_round[0] = 0
    pos = CPC
    while r < R:
        if m[r] > P:
            offs_of_round[r] = pos
            a_items.append(("A", int(pos), int(m[r])))
            pos += int(m[r])
            r += 1
        else:
            # greedy tail group of consecutive small rounds; new group when
            # padding to the group max would get wasteful
            Mstar = int(m[r])
            members = []
            gpos = pos
            while r < R and m[r] <= P and m[r] * 4 >= Mstar and len(members) < CHUNKS:
                members.append((r, int(m[r])))
                offs_of_round[r] = gpos
                gpos += Mstar
                r += 1
            g_items.append(("G", int(pos), Mstar, members))
            pos = gpos
    items = [("R0",)] + g_items + a_items
    # split the last big round so the final flush gates on one small max
    if a_items:
        kind, off, mr = a_items[-1]
        q, rem = divmod(mr, P)
        if q > 0:
            items = items[:-1]
            if rem > 0:
                items.append(("AB", off + q * P, q, rem))
            items.append(("AA", off, q))
    return items, offs_of_round, int(pos)


def _host_prep(x_locs, x_clusters, edge_src, edge_dst):
    """Build per-core bf16 round-major row streams."""
    x_locs = np.asarray(x_locs, dtype=np.float32)
    x_bf = x_locs.astype(BF16)
    src = np.asarray(edge_src).astype(np.int64)
    dst = np.asarray(edge_dst).astype(np.int64)
    n_edges = dst.shape[0]

    counts = np.bincount(dst, minlength=N_CLUSTERS)  # [32768]

    gorder = np.argsort(-counts, kind="stable")
    grank = np.empty_like(gorder)
    grank[gorder] = np.arange(N_CLUSTERS)
    order = np.ascontiguousarray(gorder.reshape(CPC, N_CORES).T)  # [8, CPC]

    # occurrence index of each edge within its dst cluster
    by_dst = np.argsort(dst, kind="stable")
    group_start = np.zeros(N_CLUSTERS, dtype=np.int64)
    np.cumsum(counts[:-1], out=group_start[1:])
    occ = np.empty(n_edges, dtype=np.int64)
    occ[by_dst] = np.arange(n_edges, dtype=np.int64) - group_start[dst[by_dst]]

    g_of = grank[dst]
    core_of = g_of % N_CORES
    rank_of = g_of // N_CORES  # local rank s, count-sorted desc

    R = max(int(counts.max()), 1)
    counts_sorted = counts[gorder]
    m_r_g = (counts_sorted[None, :] > np.arange(R)[:, None]).sum(axis=1)
    m = (m_r_g + N_CORES - 1) // N_CORES  # shared per-core round size
    m[0] = CPC  # round 0 covers every slot (zeros for empty clusters)

    items, offs_of_round, TOT = _make_schedule(m)

    # chunk-major blocks everywhere: slot within a round block == local rank
    slot = offs_of_round[occ] + rank_of

    slot_src = np.full((N_CORES, TOT), -1, dtype=np.int64)
    slot_src[core_of, slot] = src

    in_maps = []
    for k in range(N_CORES):
        ss = slot_src[k]
        stream = x_bf[np.maximum(ss, 0)]  # [TOT, 256] bf16
        pad = ss < 0
        p0 = np.flatnonzero(pad[:CPC])
        if p0.size:
            stream[p0] = 0.0  # empty clusters -> 0
        pr = np.flatnonzero(pad[CPC:]) + CPC
        if pr.size:
            stream[pr] = NEG  # pads (group + straggler) are max-neutral
        in_maps.append({"rows": np.ascontiguousarray(stream)})

    return in_maps, order, items, TOT


def _build_program(items, TOT, bufs=9):
    from concourse import bacc, mybir
    from concourse._compat import axon_active
    from concourse.tile import TileContext

    nc = bacc.Bacc(
        "TRN2",
        target_bir_lowering=False,
        debug=not axon_active(),
        num_devices=N_CORES,
    )
    rows_h = nc.dram_tensor("rows", [TOT, D], mybir.dt.bfloat16, kind="ExternalInput")
    out_h = nc.dram_tensor(
        "out", [P, CHUNKS, D], mybir.dt.bfloat16, kind="ExternalOutput"
    )

    # per-chunk last-toucher item index, for early writeback
    def touched(it):
        if it[0] == "R0":
            return range(CHUNKS)
        if it[0] == "A":
            q, rem = divmod(it[2], P)
            return range(q + (1 if rem else 0))
        if it[0] == "AB":
            return range(it[2], it[2] + 1)
        if it[0] == "AA":
            return range(it[2])
        return range(1)  # G: chunk 0 only

    last_touch = [0] * CHUNKS
    for i, it in enumerate(items):
        for c in touched(it):
            last_touch[c] = i

    with TileContext(nc) as tc:
        with (
            tc.tile_pool(name="accp", bufs=1) as accp,
            tc.tile_pool(name="stagep", bufs=bufs) as stagep,
        ):
            acc = accp.tile([P, CHUNKS * D], mybir.dt.bfloat16)
            acc3 = acc[:].rearrange("p (x f) -> p x f", f=D)
            pending_hi = CHUNKS

            def flush(i):
                nonlocal pending_hi
                lo = pending_hi
                while lo > 0 and last_touch[lo - 1] <= i:
                    lo -= 1
                # merge tiny writebacks mid-stream, but flush eagerly near
                # the end so the final flush gates on as little as possible
                eager = i >= len(items) - 4
                if lo < pending_hi and (eager or pending_hi - lo >= 3):
                    # scalar-queue issue: a DMA's sem wait holds its SEQ, so
                    # putting writebacks on SP would stall stream-DMA issue
                    nc.scalar.dma_start(
                        out=out_h.ap()[:, lo:pending_hi, :],
                        in_=acc3[:, lo:pending_hi, :],
                    )
                    pending_hi = lo

            def dma_a_piece(dst, off, q):
                blk = rows_h.ap()[off : off + q * P].rearrange(
                    "(x p) f -> p x f", p=P
                )
                dst3 = dst[:, : q * D].rearrange("p (x f) -> p x f", f=D)
                nc.sync.dma_start(out=dst3, in_=blk)

            def dma_b_piece(dst, off, q, rem):
                nc.sync.dma_start(
                    out=dst[0:rem, q * D : (q + 1) * D],
                    in_=rows_h.ap()[off : off + rem],
                )

            for i, it in enumerate(items):
                if it[0] == "R0":
                    dma_a_piece(acc, 0, CHUNKS)
                elif it[0] == "A":
                    _, off, mr = it
                    q, rem = divmod(mr, P)
                    st = stagep.tile([P, CHUNKS * D], mybir.dt.bfloat16, tag="st")
                    if q > 0:
                        dma_a_piece(st, off, q)
                    if rem > 0:
                        dma_b_piece(st, off + q * P, q, rem)
                    if q > 0:
                        w = q * D
                        nc.vector.tensor_max(
                            out=acc[:, :w], in0=acc[:, :w], in1=st[:, :w]
                        )
                    if rem > 0:
                        sl = slice(q * D, (q + 1) * D)
                        nc.vector.tensor_max(
                            out=acc[0:rem, sl], in0=acc[0:rem, sl], in1=st[0:rem, sl]
                        )
                elif it[0] == "AB":
                    _, off, q, rem = it
                    st = stagep.tile([P, CHUNKS * D], mybir.dt.bfloat16, tag="st")
                    dma_b_piece(st, off, q, rem)
                    sl = slice(q * D, (q + 1) * D)
                    nc.vector.tensor_max(
                        out=acc[0:rem, sl], in0=acc[0:rem, sl], in1=st[0:rem, sl]
                    )
                elif it[0] == "AA":
                    _, off, q = it
                    st = stagep.tile([P, CHUNKS * D], mybir.dt.bfloat16, tag="st")
                    dma_a_piece(st, off, q)
                    w = q * D
                    nc.vector.tensor_max(
                        out=acc[:, :w], in0=acc[:, :w], in1=st[:, :w]
                    )
                else:  # G
                    _, off, Mstar, members = it
                    T = len(members)
                    st = stagep.tile([P, CHUNKS * D], mybir.dt.bfloat16, tag="st")
                    blk = rows_h.ap()[off : off + T * Mstar].rearrange(
                        "(t p) f -> p t f", p=Mstar
                    )
                    st3 = st[0:Mstar, : T * D].rearrange("p (t f) -> p t f", f=D)
                    nc.sync.dma_start(out=st3, in_=blk)
                    for t, (r, mr) in enumerate(members):
                        nc.vector.tensor_max(
                            out=acc[0:mr, 0:D],
                            in0=acc[0:mr, 0:D],
                            in1=st[0:mr, t * D : (t + 1) * D],
                        )
                flush(i)
    nc.compile()
    return nc


def kernel(x_locs, x_clusters, edge_src, edge_dst):
    global LAST_RESULTS, LAST_NC
    from concourse.bass_utils import run_bass_kernel_spmd

    x_clusters = np.ascontiguousarray(np.asarray(x_clusters, dtype=np.float32))
    in_maps, order, items, TOT = _host_prep(x_locs, x_clusters, edge_src, edge_dst)
    nc = _build_program(items, TOT)
    LAST_NC = nc
    try:
        res = run_bass_kernel_spmd(nc, in_maps, list(range(N_CORES)))
    except Exception:
        # transient NRT/tunnel faults clear on re-execution; retry once
        res = run_bass_kernel_spmd(nc, in_maps, list(range(N_CORES)))
    LAST_RESULTS = res

    full = np.empty((N_CLUSTERS, 2 * D), dtype=np.float32)
    full[:, :D] = x_clusters
    for k in range(N_CORES):
        o = np.asarray(res.results[k]["out"])  # [P, CHUNKS, D] bf16
        o = o.transpose(1, 0, 2).reshape(CPC, D)  # indexed by sorted rank
        full[order[k], D:] = o.astype(np.float32)
    return full
